# revision 3
# baseline (speedup 1.0000x reference)
"""Bass/Tile kernel v2 for the pre-LN attention block (dense_transformer).

Sharding: 8 cores = 4 batches x 2 query-halves (host rotates rows so each
core's rows are [0, NQ)).

v2 changes vs baseline:
  - fused A+B: per 1024-key chunk, LN1 -> fp8 transpose -> K/Q/V matmuls,
    so PE has work from the start
  - QKV in fp8e4m3 with DoubleRow perf mode (pairs of dt-tiles, 256-col
    psum chunks); k/q/v outputs stay bf16
  - V bias + proj bias folded into host-prebiased xb tensor (no ones-row
    matmuls for V/proj); K/Q biases fused into DVE psum->sbuf copies
  - w1/w2/wproj in bf16, SBUF-resident (w1+wproj preloaded at start,
    w2 during D) -> no DMA stalls in MLP
  - D transposes in bf16 (ident_bf), h2T/gT bf16; proj runs bf16 x bf16
    via a bf16 normalized waTb
"""

import sys

sys.path.insert(0, "/opt/trn_rl_repo")

from contextlib import ExitStack

import numpy as np
import ml_dtypes

import concourse.bass as bass
import concourse.tile as tile
import concourse.mybir as mybir
from concourse import bacc

F32 = mybir.dt.float32
F32R = mybir.dt.float32r
BF16 = mybir.dt.bfloat16
FP8 = mybir.dt.float8e4
AF = mybir.ActivationFunctionType
ALU = mybir.AluOpType
DRM = mybir.MatmulPerfMode.DoubleRow

DIM = 768
H = 12
DH = 64
HID = 3072
SCALE = DH ** -0.5
EPS = 1e-6
P = 128
DT = DIM // P
DP = DT // 2  # dt pairs for DoubleRow


def r(x):
    return x.bitcast(F32R)


def build_nc(S=2048, NQ=1024, gelu=True, repeat=1, stop_after=None, fp8_qkv=True):
    KT = S // P
    NQT = NQ // P
    QC = NQ // 512
    HT = HID // P
    CH = 1024  # keys per fused A+B chunk
    CT = CH // P  # tiles per chunk
    NCH = S // CH
    HDT = FP8 if fp8_qkv else BF16

    nc = bacc.Bacc("TRN2", target_bir_lowering=False, debug=False, num_devices=8)

    dx = nc.dram_tensor("x", [P, KT, DIM], BF16, kind="ExternalInput").ap()
    dxb = nc.dram_tensor("xb", [P, NQT, DIM], BF16, kind="ExternalInput").ap()
    dmask = nc.dram_tensor("maskT", [P, KT, NQ], BF16, kind="ExternalInput").ap()
    dwv = nc.dram_tensor("wvr", [P, DT, DIM], HDT, kind="ExternalInput").ap()
    dwkq = nc.dram_tensor("wkqr", [2 * DT, P, DT * P], HDT, kind="ExternalInput").ap()
    dbqkv = nc.dram_tensor("bqkv_pp", [P, 3 * DT], F32, kind="ExternalInput").ap()
    dwproj = nc.dram_tensor("wprojr", [P, DT, DIM], BF16, kind="ExternalInput").ap()
    dw1 = nc.dram_tensor("w1r", [HT, P, DT * P], BF16, kind="ExternalInput").ap()
    db1 = nc.dram_tensor("b1_pp", [P, HT], F32, kind="ExternalInput").ap()
    dw2 = nc.dram_tensor("w2", [HID, DIM], BF16, kind="ExternalInput").ap()
    db2 = nc.dram_tensor("b2_row", [1, DIM], BF16, kind="ExternalInput").ap()
    dones = nc.dram_tensor("ones_row", [1, P], BF16, kind="ExternalInput").ap()
    dident8 = nc.dram_tensor("ident_h", [P, P], HDT, kind="ExternalInput").ap()
    dident_bf = nc.dram_tensor("ident_bf", [P, P], BF16, kind="ExternalInput").ap()
    dy = nc.dram_tensor("y", [P, NQT, DIM], F32, kind="ExternalOutput").ap()

    with nc.allow_low_precision(
        reason="fp8 qkv + bf16 attention/mlp validated against 2e-2 gate"
    ), tile.TileContext(nc) as tc, ExitStack() as top:
        rep_ctx = tc.For_i(0, repeat, 1) if repeat > 1 else ExitStack()
        top.enter_context(rep_ctx)
        consts = top.enter_context(tc.tile_pool(name="consts", bufs=1))
        ident8 = consts.tile([P, P], HDT)
        nc.gpsimd.dma_start(out=ident8[:], in_=dident8[:])
        ident_bf = consts.tile([P, P], BF16)
        nc.gpsimd.dma_start(out=ident_bf[:], in_=dident_bf[:])
        ones_col = consts.tile([1, P], BF16)
        nc.gpsimd.dma_start(out=ones_col[:], in_=dones[:])
        eps_t = consts.tile([P, 1], F32)
        nc.vector.memset(eps_t[:], EPS)
        bqkv_pp = consts.tile([P, 3 * DT], F32)
        nc.gpsimd.dma_start(out=bqkv_pp[:], in_=dbqkv[:])
        b1_pp = consts.tile([P, HT], F32)
        nc.gpsimd.dma_start(out=b1_pp[:], in_=db1[:])
        b2_row = consts.tile([1, DIM], BF16)
        nc.gpsimd.dma_start(out=b2_row[:], in_=db2[:])

        def layer_norm_tile(stats_pool, x_ap, out_ap):
            stats = stats_pool.tile([P, 2, 6], F32, tag="lnstats")
            for sg in range(2):
                nc.vector.bn_stats(
                    out=stats[:, sg, :], in_=x_ap[:, sg * 384 : (sg + 1) * 384]
                )
            mv = stats_pool.tile([P, 2], F32, tag="lnmv")
            nc.vector.bn_aggr(out=mv[:], in_=stats[:])
            sd = stats_pool.tile([P, 1], F32, tag="lnsd")
            nc.scalar.activation(
                out=sd[:], in_=mv[:, 1:2], func=AF.Sqrt, bias=eps_t[:], scale=1.0
            )
            rstd = stats_pool.tile([P, 1], F32, tag="lnrstd")
            nc.vector.reciprocal(out=rstd[:], in_=sd[:])
            nc.vector.tensor_scalar(
                out=out_ap,
                in0=x_ap,
                scalar1=mv[:, 0:1],
                scalar2=rstd[:],
                op0=ALU.subtract,
                op1=ALU.mult,
            )

        wa_pool = top.enter_context(tc.tile_pool(name="wa", bufs=1))
        waTbs = [wa_pool.tile([P, NQ], BF16, name=f"waTb{j}") for j in range(DT)]

        w1_pool = top.enter_context(tc.tile_pool(name="w1p", bufs=1))
        w1_sb = w1_pool.tile([P, HT, DT * P], BF16, name="w1_sb") if fp8_qkv else None
        wproj_pool = top.enter_context(tc.tile_pool(name="wpp", bufs=1))
        wproj_sb = wproj_pool.tile([P, DT, DIM], BF16)

        def phase_ab(kqv_pool):
            mask_sb = kqv_pool.tile([P, KT, NQ], BF16)
            k_sb = kqv_pool.tile([P, DT, S], BF16)
            q_sb = kqv_pool.tile([P, DT, NQ], BF16)
            v_sb = kqv_pool.tile([P, KT, H * (DH + 1)], BF16)
            v4 = v_sb.rearrange("p t (h s) -> p t h s", s=DH + 1)
            nc.vector.memset(v4[:, :, :, DH : DH + 1], 1.0)

            with ExitStack() as ph:
                wq_pool = ph.enter_context(tc.tile_pool(name="wq8", bufs=1))
                wkq8 = wq_pool.tile([P, 2 * DT, DT * P], HDT)
                nc.gpsimd.dma_start(
                    out=wkq8[:], in_=dwkq.rearrange("m p o -> p m o")
                )
                wv8 = wq_pool.tile([P, DT, DIM], HDT)
                nc.gpsimd.dma_start(out=wv8[:], in_=dwv[:])
                # DoubleRow views: dt -> (dp, pair)
                wkq8v = wkq8.rearrange("p m (dp two o) -> p m dp two o", two=2, o=P)
                wv8v = wv8.rearrange("p (dp two) o -> p dp two o", two=2)

                ht_pool = ph.enter_context(tc.tile_pool(name="htp", bufs=1))
                hT = ht_pool.tile([P, DT, S], HDT if fp8_qkv else BF16)

                xo_pool = ph.enter_context(tc.tile_pool(name="xo", bufs=5 if fp8_qkv else 2))
                h_pool = ph.enter_context(tc.tile_pool(name="h1", bufs=9 if fp8_qkv else 8))
                st_pool = ph.enter_context(tc.tile_pool(name="st1", bufs=6))
                kv_ps = ph.enter_context(
                    tc.tile_pool(name="kvps", bufs=4, space=bass.MemorySpace.PSUM)
                )
                tp_ctx = ExitStack()
                tp_pool = tp_ctx.enter_context(
                    tc.tile_pool(name="tp1", bufs=2, space=bass.MemorySpace.PSUM)
                )

                def qkv_dr(out_ps, lhsT_view, rhs_view):
                    for dp in range(DP):
                        nc.tensor.matmul(
                            out_ps,
                            lhsT=lhsT_view(dp),
                            rhs=rhs_view(dp),
                            start=(dp == 0),
                            stop=(dp == DP - 1),
                            perf_mode=DRM,
                        )

                def qkv_bf(out_ps, lhsT_view, rhs_view):
                    for dt in range(DT):
                        nc.tensor.matmul(
                            out_ps,
                            lhsT=lhsT_view(dt),
                            rhs=rhs_view(dt),
                            start=(dt == 0),
                            stop=(dt == DT - 1),
                        )

                for cc in range(NCH):
                    hts = []
                    xos = []
                    for tp2i in range(CT // 2):
                        xo = xo_pool.tile([P, 2, DIM], BF16, tag="xo")
                        nc.sync.dma_start(
                            out=xo[:], in_=dx[:, cc * CT + 2 * tp2i : cc * CT + 2 * tp2i + 2, :]
                        )
                        xos.append(xo)
                    if cc == NCH - 1:
                        for mq in range(4):
                            nc.sync.dma_start(
                                out=mask_sb[:, mq * (KT // 4) : (mq + 1) * (KT // 4), :],
                                in_=dmask[:, mq * (KT // 4) : (mq + 1) * (KT // 4), :],
                            )
                        if w1_sb is not None:
                            nc.sync.dma_start(
                                out=w1_sb[:], in_=dw1.rearrange("h p o -> p h o")
                            )
                        nc.sync.dma_start(out=wproj_sb[:], in_=dwproj[:])
                    for t in range(CT):
                        h_t = h_pool.tile([P, DIM], BF16, tag="h")
                        layer_norm_tile(st_pool, xos[t // 2][:, t % 2, :], h_t[:])
                        hts.append(h_t)
                    ps = [
                        tp_pool.tile([P, 2, CH], BF16, tag="tp", name=f"tp_{cc}_{i}")
                        for i in range(DP)
                    ]
                    for t in range(CT):
                        for dt in range(DT):
                            nc.tensor.transpose(
                                ps[dt // 2][:, dt % 2, t * P : (t + 1) * P],
                                hts[t][:, dt * P : (dt + 1) * P],
                                ident_bf[:],
                            )
                    for dp in range(DP):
                        nc.vector.tensor_copy(
                            out=hT[:, 2 * dp : 2 * dp + 2, cc * CH : (cc + 1) * CH],
                            in_=ps[dp][:],
                        )

                tp_ctx.close()
                for cc in range(NCH):
                    hTc = hT[:, :, cc * CH : (cc + 1) * CH]
                    for m in range(DT):
                        for c4 in range(CH // 256):
                            psk = kv_ps.tile(
                                [P, 512], F32, tag="kv", name=f"psk_{cc}_{m}_{c4}"
                            )
                            cs = slice(c4 * 256, (c4 + 1) * 256)
                            if fp8_qkv:
                                qkv_dr(
                                    psk[:, 0:256],
                                    lambda dp, m=m: wkq8v[:, m, dp, :, :],
                                    lambda dp, cs=cs, hTc=hTc: hTc[:, 2 * dp : 2 * dp + 2, cs],
                                )
                            else:
                                qkv_bf(
                                    psk[:, 0:256],
                                    lambda dt, m=m: wkq8v[:, m, dt // 2, dt % 2, :],
                                    lambda dt, cs=cs, hTc=hTc: hTc[:, dt, cs],
                                )
                            nc.scalar.activation(
                                out=k_sb[:, m, cc * CH + c4 * 256 : cc * CH + (c4 + 1) * 256],
                                in_=psk[:, 0:256],
                                func=AF.Identity,
                                bias=bqkv_pp[:, DT + m : DT + m + 1],
                                scale=1.0,
                            )
                    if cc == 0:
                        for m in range(DT):
                            for c4 in range(CH // 256):
                                psq = kv_ps.tile(
                                    [P, 512], F32, tag="kv", name=f"psq_{m}_{c4}"
                                )
                                cs = slice(c4 * 256, (c4 + 1) * 256)
                                if fp8_qkv:
                                    qkv_dr(
                                        psq[:, 0:256],
                                        lambda dp, m=m: wkq8v[:, DT + m, dp, :, :],
                                        lambda dp, cs=cs, hTc=hTc: hTc[:, 2 * dp : 2 * dp + 2, cs],
                                    )
                                else:
                                    qkv_bf(
                                        psq[:, 0:256],
                                        lambda dt, m=m: wkq8v[:, DT + m, dt // 2, dt % 2, :],
                                        lambda dt, cs=cs, hTc=hTc: hTc[:, dt, cs],
                                    )
                                nc.scalar.activation(
                                    out=q_sb[:, m, c4 * 256 : (c4 + 1) * 256],
                                    in_=psq[:, 0:256],
                                    func=AF.Identity,
                                    bias=bqkv_pp[:, m : m + 1],
                                    scale=1.0,
                                )
                for cc in range(NCH):
                    hTc = hT[:, :, cc * CH : (cc + 1) * CH]
                    for t in range(CT):
                        tt = cc * CT + t
                        for c3 in range(DIM // 256):
                            psv = kv_ps.tile(
                                [P, 512], F32, tag="kv", name=f"psv_{tt}_{c3}"
                            )
                            cs = slice(c3 * 256, (c3 + 1) * 256)
                            if fp8_qkv:
                                qkv_dr(
                                    psv[:, 0:256],
                                    lambda dp, t=t, hTc=hTc: hTc[:, 2 * dp : 2 * dp + 2, t * P : (t + 1) * P],
                                    lambda dp, cs=cs: wv8v[:, dp, :, cs],
                                )
                            else:
                                qkv_bf(
                                    psv[:, 0:256],
                                    lambda dt, t=t, hTc=hTc: hTc[:, dt, t * P : (t + 1) * P],
                                    lambda dt, cs=cs: wv8v[:, dt // 2, dt % 2, cs],
                                )
                            if c3 % 2 == 0:
                                nc.vector.tensor_copy(
                                    out=v4[:, tt, c3 * 4 : (c3 + 1) * 4, 0:DH],
                                    in_=psv[:, 0:256].rearrange("p (h s) -> p h s", s=DH),
                                )
                            else:
                                nc.scalar.copy(
                                    out=v4[:, tt, c3 * 4 : (c3 + 1) * 4, 0:DH],
                                    in_=psv[:, 0:256].rearrange("p (h s) -> p h s", s=DH),
                                )
            return mask_sb, k_sb, q_sb, v_sb, v4

        def phase_c(mask_sb, k_sb, q_sb, v_sb):
            with ExitStack() as ph:
                s_ps = ph.enter_context(
                    tc.tile_pool(name="sps", bufs=2, space=bass.MemorySpace.PSUM)
                )
                av_ps = ph.enter_context(
                    tc.tile_pool(name="avps", bufs=2, space=bass.MemorySpace.PSUM)
                )
                p_pool = ph.enter_context(tc.tile_pool(name="pp", bufs=7))
                den_pool = ph.enter_context(tc.tile_pool(name="den", bufs=2))
                scr_pool = ph.enter_context(tc.tile_pool(name="scr", bufs=2))
                rb_pool = ph.enter_context(tc.tile_pool(name="rb", bufs=3))
                pending_norm = []

                def emit_norm(last=False):
                    while pending_norm:
                        jj, rbjj = pending_norm.pop(0)
                        scr_j = scr_pool.tile(
                            [P, NQ], F32, tag="scr", name=f"scr_{jj}"
                        )
                        nc.vector.reciprocal_approx_accurate(
                            out=rbjj[:], in_=rbjj[:], scratch=scr_j[:]
                        )
                        for hh in range(2):
                            nc.vector.tensor_tensor(
                                out=waTbs[jj][hh * DH : (hh + 1) * DH, :],
                                in0=waTbs[jj][hh * DH : (hh + 1) * DH, :],
                                in1=rbjj[hh * DH : (hh + 1) * DH, :],
                                op=ALU.mult,
                            )
                JORD = [1, 2, 3, 4, 5, 0]
                for ji, j in enumerate(JORD):
                    avs = [
                        av_ps.tile([P, NQ], F32, tag="av", name=f"av_{j}_{i}")
                        for i in range(2)
                    ]
                    for kt in range(KT):
                        pts = []
                        for hh in range(2):
                            lo, hi = hh * DH, (hh + 1) * DH
                            pss = s_ps.tile([P, NQ], F32, tag="s")
                            for c in range(QC):
                                nc.tensor.matmul(
                                    pss[:, c * 512 : (c + 1) * 512],
                                    lhsT=k_sb[lo:hi, j, kt * P : (kt + 1) * P],
                                    rhs=q_sb[lo:hi, j, c * 512 : (c + 1) * 512],
                                    tile_position=(lo, 0),
                                )
                            pe_t = p_pool.tile([P, NQ], BF16, tag="pe")
                            nc.scalar.activation(
                                out=pe_t[:], in_=pss[:], func=AF.Exp, scale=SCALE
                            )
                            pt = p_pool.tile([P, NQ], BF16, tag="p")
                            nc.vector.tensor_tensor(
                                out=pt[:], in0=pe_t[:], in1=mask_sb[:, kt, :], op=ALU.mult
                            )
                            pts.append(pt)
                        for hh in range(2):
                            hgl = (2 * j + hh) * (DH + 1)
                            for c in range(QC):
                                nc.tensor.matmul(
                                    avs[hh][0 : DH + 1, c * 512 : (c + 1) * 512],
                                    lhsT=v_sb[:, kt, hgl : hgl + DH + 1],
                                    rhs=pts[hh][:, c * 512 : (c + 1) * 512],
                                    start=(kt == 0),
                                    stop=(kt == KT - 1),
                                )
                    emit_norm()
                    rbj = rb_pool.tile([P, NQ], F32, tag="rb", name=f"rb_{j}")
                    sc2 = scr_pool.tile([P, NQ], F32, tag="scr", name=f"sc2_{j}")
                    for hh in range(2):
                        den_h = den_pool.tile([1, NQ], F32, tag="den", name=f"den_{j}_{hh}")
                        nc.vector.tensor_copy(
                            out=den_h[0:1, :], in_=avs[hh][DH : DH + 1, :]
                        )
                        if hh == 0:
                            nc.gpsimd.partition_broadcast(
                                rbj[0:DH, :], den_h[0:1, :], channels=DH
                            )
                        else:
                            nc.gpsimd.partition_broadcast(
                                sc2[0:DH, :], den_h[0:1, :], channels=DH
                            )
                            nc.vector.tensor_copy(
                                out=rbj[DH : 2 * DH, :], in_=sc2[0:DH, :]
                            )
                    for hh in range(2):
                        nc.vector.tensor_copy(
                            out=waTbs[j][hh * DH : (hh + 1) * DH, :],
                            in_=avs[hh][0:DH, :],
                        )
                    pending_norm.append((j, rbj))
                    if ji == H // 2 - 1:
                        emit_norm(last=True)

        def phase_de():
            w1s = w1_sb
            if w1s is None:
                w1_pool2 = top.enter_context(tc.tile_pool(name="w1p2", bufs=1))
                w1s = w1_pool2.tile([P, HT, DT * P], BF16)
                nc.gpsimd.dma_start(out=w1s[:], in_=dw1.rearrange("h p o -> p h o"))
            w2_pool = top.enter_context(tc.tile_pool(name="w2p", bufs=1))
            w2_sb = w2_pool.tile([P, HT, DIM], BF16)
            nc.gpsimd.dma_start(
                out=w2_sb[:], in_=dw2.rearrange("(h p) o -> p h o", p=P)
            )
            de_pool = top.enter_context(tc.tile_pool(name="de", bufs=1))
            x2_sb = de_pool.tile([P, NQT, DIM], F32)
            h2T = de_pool.tile([P, DT, NQ], BF16)

            # ---- phase D ----
            with ExitStack() as ph:
                xr_pool = ph.enter_context(tc.tile_pool(name="xr", bufs=3))
                pr_ps = ph.enter_context(
                    tc.tile_pool(name="prps", bufs=2, space=bass.MemorySpace.PSUM)
                )
                h2_pool = ph.enter_context(tc.tile_pool(name="h2", bufs=6))
                st2_pool = ph.enter_context(tc.tile_pool(name="st2", bufs=6))
                tp2_pool = ph.enter_context(
                    tc.tile_pool(name="tp2", bufs=3, space=bass.MemorySpace.PSUM)
                )
                for half in range(2):
                    h2s = []
                    for t4 in range(4):
                        t = half * 4 + t4
                        xr = xr_pool.tile([P, DIM], BF16, tag="xr")
                        nc.sync.dma_start(out=xr[:], in_=dxb[:, t, :])
                        psp = pr_ps.tile([P, DIM], F32, tag="pr")
                        for c0, cw in ((0, 512), (512, 256)):
                            for di, dt in enumerate([1, 2, 3, 4, 5, 0]):
                                nc.tensor.matmul(
                                    psp[:, c0 : c0 + cw],
                                    lhsT=waTbs[dt][:, t * P : (t + 1) * P],
                                    rhs=wproj_sb[:, dt, c0 : c0 + cw],
                                    start=(di == 0),
                                    stop=(di == DT - 1),
                                )
                        nc.vector.tensor_tensor(
                            out=x2_sb[:, t, :], in0=psp[:], in1=xr[:], op=ALU.add
                        )
                        h2_t = h2_pool.tile([P, DIM], BF16, tag="h2")
                        layer_norm_tile(st2_pool, x2_sb[:, t, :], h2_t[:])
                        h2s.append(h2_t)
                    ps2 = [
                        tp2_pool.tile(
                            [P, 2, 512], BF16, tag="tp2", name=f"tp2_{half}_{i}"
                        )
                        for i in range(DP)
                    ]
                    for t4 in range(4):
                        for dt in range(DT):
                            nc.tensor.transpose(
                                ps2[dt // 2][:, dt % 2, t4 * P : (t4 + 1) * P],
                                h2s[t4][:, dt * P : (dt + 1) * P],
                                ident_bf[:],
                            )
                    for dp in range(DP):
                        nc.vector.tensor_copy(
                            out=h2T[
                                :, 2 * dp : 2 * dp + 2, half * 512 : (half + 1) * 512
                            ],
                            in_=ps2[dp][:],
                        )

            # ---- phase E ----
            with ExitStack() as ph:
                g_pool = ph.enter_context(tc.tile_pool(name="gp", bufs=1))
                f_ps = ph.enter_context(
                    tc.tile_pool(name="fps", bufs=3, space=bass.MemorySpace.PSUM)
                )
                y_ps = ph.enter_context(
                    tc.tile_pool(name="yps", bufs=4, space=bass.MemorySpace.PSUM)
                )
                y_pool = ph.enter_context(tc.tile_pool(name="yp", bufs=2))

                MC = NQ // 512
                NTC = 512 // P
                for mc in range(MC):
                    q0 = mc * 512
                    gT = g_pool.tile([P, HT, 512], BF16, tag="g", name=f"g_{mc}")
                    for ht in range(HT):
                        psf = f_ps.tile([P, 512], F32, tag="f", name=f"psf_{mc}_{ht}")
                        for dt in range(DT):
                            nc.tensor.matmul(
                                psf[:],
                                lhsT=w1s[:, ht, dt * P : (dt + 1) * P],
                                rhs=h2T[:, dt, q0 : q0 + 512],
                                start=(dt == 0),
                                stop=(dt == DT - 1),
                            )
                        nc.scalar.activation(
                            out=gT[:, ht, :],
                            in_=psf[:],
                            func=AF.Gelu if gelu else AF.Identity,
                            bias=b1_pp[:, ht : ht + 1],
                            scale=1.0,
                        )
                    y_mc = y_pool.tile(
                        [P, NTC, DIM], F32, tag="yt", name=f"yt_{mc}"
                    )
                    for c0, cw in ((0, 512), (512, 256)):
                        psy = [
                            y_ps.tile([P, 512], F32, tag="y", name=f"psy_{mc}_{c0}_{i}")
                            for i in range(NTC)
                        ]
                        for ht in range(HT):
                            for t in range(NTC):
                                nc.tensor.matmul(
                                    psy[t][:, :cw],
                                    lhsT=gT[:, ht, t * P : (t + 1) * P],
                                    rhs=w2_sb[:, ht, c0 : c0 + cw],
                                    start=(ht == 0),
                                    stop=False,
                                )
                        for t in range(NTC):
                            nc.tensor.matmul(
                                psy[t][:, :cw],
                                lhsT=ones_col[:, :],
                                rhs=b2_row[:, c0 : c0 + cw],
                                start=False,
                                stop=True,
                            )
                        for t in range(NTC):
                            tg = mc * NTC + t
                            nc.vector.tensor_tensor(
                                out=y_mc[:, t, c0 : c0 + cw],
                                in0=psy[t][:, :cw],
                                in1=x2_sb[:, tg, c0 : c0 + cw],
                                op=ALU.add,
                            )
                    nc.sync.dma_start(
                        out=dy[:, mc * NTC : (mc + 1) * NTC, :], in_=y_mc[:]
                    )

        with ExitStack() as s_kqv:
            kqv_pool = s_kqv.enter_context(tc.tile_pool(name="kqv", bufs=1))
            mask_sb, k_sb, q_sb, v_sb, v4 = phase_ab(kqv_pool)
            if stop_after != "ab":
                phase_c(mask_sb, k_sb, q_sb, v_sb)
        if stop_after is None:
            phase_de()
        else:
            _store_dummy(nc, tc, dy, NQT)

    nc.compile()
    return nc


def _store_dummy(nc, tc, dy, NQT):
    with ExitStack() as s_dummy:
        dpool = s_dummy.enter_context(tc.tile_pool(name="dumy", bufs=1))
        dt_ = dpool.tile([P, DIM], F32)
        nc.vector.memset(dt_[:], 0.0)
        for t in range(NQT):
            nc.sync.dma_start(out=dy[:, t, :], in_=dt_[:])


# ---------------- host-side preprocessing ----------------


def make_core_inputs(inp, core, S=2048, NQ=1024, fp8_qkv=True):
    b, half = core // 2, core % 2
    q0 = half * NQ
    x = np.asarray(inp["x"][b], np.float32)
    xrot = np.concatenate([x[q0 : q0 + NQ], x[:q0] if q0 else x[NQ:]], axis=0)
    mask = np.asarray(inp["mask"][b, 0], np.float32)
    mq = mask[q0 : q0 + NQ]
    mrot = np.concatenate(
        [mq[:, q0 : q0 + NQ], mq[:, :q0] if q0 else mq[:, NQ:]], axis=1
    )
    maskT = np.ascontiguousarray(mrot.T).astype(ml_dtypes.bfloat16)

    g1 = np.asarray(inp["g1"], np.float32)
    be1 = np.asarray(inp["beta1"], np.float32)
    g2 = np.asarray(inp["g2"], np.float32)
    be2 = np.asarray(inp["beta2"], np.float32)
    w_qkv = np.asarray(inp["w_qkv"], np.float32)
    w_proj = np.asarray(inp["w_proj"], np.float32)
    wqkv = w_qkv * g1[:, None]
    bqkv = np.asarray(inp["b_qkv"], np.float32) + be1 @ w_qkv
    bv = bqkv[2 * DIM :]
    bproj_eff = np.asarray(inp["b_proj"], np.float32) + bv @ w_proj
    w1f = np.asarray(inp["w1"], np.float32)
    w1 = w1f * g2[:, None]
    b1 = np.asarray(inp["b1"], np.float32) + be2 @ w1f

    KTl, NQTl = S // P, NQ // P
    xpk = np.ascontiguousarray(xrot.reshape(KTl, P, DIM).transpose(1, 0, 2)).astype(
        ml_dtypes.bfloat16
    )
    xb = xrot[:NQ] + bproj_eff[None, :]
    xbpk = np.ascontiguousarray(xb.reshape(NQTl, P, DIM).transpose(1, 0, 2)).astype(
        ml_dtypes.bfloat16
    )
    mpk = np.ascontiguousarray(maskT.reshape(KTl, P, NQ).transpose(1, 0, 2))
    HNP = ml_dtypes.float8_e4m3fn if fp8_qkv else ml_dtypes.bfloat16
    wq8 = wqkv.astype(HNP)
    wvr = np.ascontiguousarray(
        wq8[:, 2 * DIM :].reshape(DT, P, DIM).transpose(1, 0, 2)
    )
    wkqr = np.zeros((2 * DT, P, DT * P), HNP)
    for m in range(DT):
        wkqr[m] = (
            wq8[:, DIM + m * P : DIM + (m + 1) * P]
            .reshape(DT, P, P).transpose(1, 0, 2).reshape(P, DT * P)
        )
        wkqr[DT + m] = (
            wq8[:, m * P : (m + 1) * P]
            .reshape(DT, P, P).transpose(1, 0, 2).reshape(P, DT * P)
        )
    wprojr = np.ascontiguousarray(
        w_proj.reshape(DT, P, DIM).transpose(1, 0, 2)
    ).astype(ml_dtypes.bfloat16)
    HTl = HID // P
    w1r = np.ascontiguousarray(
        w1.reshape(DT, P, HTl, P).transpose(2, 1, 0, 3).reshape(HTl, P, DT * P)
    ).astype(ml_dtypes.bfloat16)
    return {
        "x": xpk,
        "xb": xbpk,
        "maskT": mpk,
        "wvr": wvr,
        "wkqr": wkqr,
        "bqkv_pp": np.ascontiguousarray(bqkv.reshape(3 * DIM // P, P).T),
        "wprojr": wprojr,
        "w1r": w1r,
        "b1_pp": np.ascontiguousarray(b1.reshape(HID // P, P).T),
        "w2": np.asarray(inp["w2"], np.float32).astype(ml_dtypes.bfloat16),
        "b2_row": np.asarray(inp["b2"], np.float32).reshape(1, DIM).astype(ml_dtypes.bfloat16),
        "ones_row": np.ones((1, P), ml_dtypes.bfloat16),
        "ident_h": np.eye(P, dtype=HNP),
        "ident_bf": np.eye(P, dtype=ml_dtypes.bfloat16),
    }


def assemble_output(results, B=4, S=2048, NQ=1024):
    y = np.zeros((B, S, DIM), np.float32)
    for core, res in enumerate(results):
        b, half = core // 2, core % 2
        yr = res["y"].reshape(P, NQ // P, DIM).transpose(1, 0, 2).reshape(NQ, DIM)
        y[b, half * NQ : (half + 1) * NQ] = yr
    return y


# ---------------- harness entry point ----------------

_NC_CACHE = {}


def _get_nc():
    if "nc" not in _NC_CACHE:
        _NC_CACHE["nc"] = build_nc(gelu=True)
    return _NC_CACHE["nc"]


def kernel(**inputs):
    """Full (unsharded) inputs -> full (4, 2048, 768) float32 output."""
    from concourse.bass_utils import run_bass_kernel_spmd

    nc = _get_nc()
    in_maps = [make_core_inputs(inputs, c) for c in range(8)]
    res = run_bass_kernel_spmd(nc, in_maps, core_ids=list(range(8)))
    return assemble_output(res.results)
